# revision 18
# baseline (speedup 1.0000x reference)
"""InterfaceBoundaryLoss Trainium2 kernel (v3).

Data-parallel over batch across 8 NeuronCores.  The [H,W] interface mask is
covered on the host with boxes hugging the circle: 128-row-tall boxes where
the arc is steep, 32- or 16-row bands elsewhere (cost-driven), wide clusters
split into <=64-col pieces.  Short boxes pack into 128-partition stacks of
equal quantized width; stacks of the same width form a "class" processed by
single multi-stack (4D-AP) vector instructions, so the instruction count
stays small.  phi1/phi2 are packed host-side t-major ([2,BPC,H,W]) so each
box loads with one 3D-AP DMA and psi/df are flat subtractions.

Math per mask cell: pot += (phi1-phi2)^2, der += (A*Dx(psi) + B*Dy(psi))^2
with psi = 0.025*phi2 - phi1, A = 40000*m*nx, B = 40000*m*ny shifted one
col left (layout col k = cell k+1).  Dy runs on the TensorEngine via a
banded +/-1 matrix with the moving tensor shifted one col right, so B*dy
aligns with A*Dx without extra shifts; block-halo rows are masked.  The
pot path (df, mask-mul) runs on GpSimd/DVE; one Square+accum activation
per class-half sums pot+der together.  Host sums partials in float64.
"""

import sys

for _p in ("/opt/trn_rl_repo",):
    if _p not in sys.path:
        sys.path.append(_p)

import numpy as np
import ml_dtypes

B, H, W = 64, 1024, 1024
EPS1, EPS2 = 80.0, 2.0
DX, DY = 0.001, 0.001
CX, CY = 512.0, 512.0
WEIGHT = 1.0
N_CORES = 8
BPC = B // N_CORES

TALL = 128
TALL_MAX_W = 64
WQ = (16, 32, 48, 64)
GAP = 6
SUB_W = 64
BOX_PENALTY = 2500  # cells-equivalent cost of one extra box (DMA+sem)

TRACE = False
LAST_EXEC_NS = None


class _Box:
    __slots__ = ("r0", "nr", "c0", "w", "sel", "part0", "f0")

    def __init__(self, r0, nr, c0, w):
        self.r0, self.nr, self.c0, self.w = int(r0), int(nr), int(c0), int(w)
        self.sel = None


def _clusters(cols, gap=GAP):
    out = []
    s = p = cols[0]
    for c in cols[1:]:
        if c - p > gap:
            out.append((s, p))
            s = c
        p = c
    out.append((s, p))
    return out


def _band_pieces(cols):
    """Split a band's mask cols into quantized-width pieces.
    Returns list of (pa, pb, wq, c0)."""
    out = []
    for ca, cb in _clusters(cols):
        span = cb - ca + 1
        net = SUB_W - 4
        npieces = max(1, -(-span // net)) if span > SUB_W - 2 else 1
        for pi in range(npieces):
            pa = ca + pi * net
            pb = min(pa + net - 1, cb)
            if pa > cb:
                break
            ww = pb - pa + 3
            wq = next((q for q in WQ if q >= ww), None)
            if wq is None:
                wq = -(-ww // 64) * 64
            c0 = pa - 1 - (wq - ww) // 2
            c0 = max(0, min(c0, W - wq))
            out.append((pa, pb, wq, c0))
    return out


def _plan(mask):
    h, w_ = mask.shape
    border = np.zeros_like(mask)
    border[0, :] = border[-1, :] = True
    border[:, 0] = border[:, -1] = True
    host_cells = mask & border
    core = mask & ~border
    assigned = np.zeros_like(mask)

    rows_any = np.flatnonzero(core.any(axis=1))
    boxes = []
    if len(rows_any) == 0:
        return boxes, host_cells

    def emit(r, nr):
        own_lo, own_hi = r, min(r + nr - 2, int(rows_any[-1]) + 1)
        sub = core[own_lo:own_hi]
        cols = np.flatnonzero(sub.any(axis=0))
        for pa, pb, wq, c0 in _band_pieces(cols):
            bx = _Box(r - 1, nr, c0, wq)
            sel = np.zeros((nr, wq), dtype=bool)
            s = (
                core[own_lo:own_hi, pa : pb + 1]
                & ~assigned[own_lo:own_hi, pa : pb + 1]
            )
            sel[own_lo - bx.r0 : own_hi - bx.r0, pa - c0 : pb + 1 - c0] = s
            assigned[own_lo:own_hi, pa : pb + 1] |= s
            rr_, cc_ = np.nonzero(sel)
            if len(rr_) == 0:
                continue
            assert rr_.min() >= 1 and rr_.max() <= nr - 2
            assert cc_.min() >= 1 and cc_.max() <= wq - 2
            bx.sel = sel
            boxes.append(bx)
        return own_hi

    def band_cost(r, nr):
        own_lo, own_hi = r, min(r + nr - 2, int(rows_any[-1]) + 1)
        cols = np.flatnonzero(core[own_lo:own_hi].any(axis=0))
        if len(cols) == 0:
            return 0.0, 0
        pieces = _band_pieces(cols)
        return sum(nr * wq + BOX_PENALTY for _, _, wq, _ in pieces), own_hi

    r = int(rows_any[0])
    rmax = int(rows_any[-1])
    while r <= rmax:
        if not core[r].any():
            r += 1
            continue
        # tall band if clusters stay narrow over 126 owned rows
        own_hi = min(r + TALL - 2, rmax + 1)
        cols = np.flatnonzero(core[r:own_hi].any(axis=0))
        cls = _clusters(cols) if len(cols) else []
        if (
            cls
            and max(cb - ca + 1 for ca, cb in cls) <= TALL_MAX_W
            and own_hi - r >= 96
            and r - 1 + TALL <= h
        ):
            r = emit(r, TALL)
            continue
        # otherwise 32-row band (SBUF APs must start at a multiple of 32,
        # so shorter bands cannot pack the partition dim)
        r = emit(r, 32)

    leftover = core & ~assigned
    if leftover.any():
        host_cells = host_cells | leftover
    return boxes, host_cells


def _stack(boxes):
    """Pack boxes into 128-partition stacks of equal width (class = width).
    Talls stand alone.  Returns ordered stack list grouped by class, and
    per-class lists; assigns part0/f0."""
    by_w = {}
    for bx in boxes:
        by_w.setdefault(bx.w, []).append(bx)
    classes = []  # (w, [stacks])
    for wq in sorted(by_w, key=lambda w: -w):
        stacks = []
        cur, used = [], 0
        # first-fit in emit order keeps vertically-adjacent bands together
        for bx in by_w[wq]:
            if used + bx.nr > 128:
                stacks.append(cur)
                cur, used = [], 0
            bx.part0 = used
            cur.append(bx)
            used += bx.nr
        if cur:
            stacks.append(cur)
        classes.append((wq, stacks))
    f = 0
    ordered = []
    for wq, stacks in classes:
        for st in stacks:
            for bx in st:
                bx.f0 = f
            ordered.append(st)
            f += wq
    return classes, ordered, f


def _normals(h, w):
    ii = np.arange(h, dtype=np.float64)[:, None]
    jj = np.arange(w, dtype=np.float64)[None, :]
    nx = jj - CX
    ny = ii - CY
    norm = np.sqrt(nx * nx + ny * ny)
    safe = np.where(norm > 0, norm, 1.0)
    return nx / safe, ny / safe


def _host_contrib(cells_ij, phi1, phi2, nx, ny):
    if len(cells_ij[0]) == 0:
        return 0.0
    ii, jj = cells_ij
    p1 = phi1.astype(np.float64)
    p2 = phi2.astype(np.float64)
    d = p1[:, ii, jj] - p2[:, ii, jj]
    tot = float(np.sum(d * d))
    jc = np.clip(jj, 1, W - 2)
    ic = np.clip(ii, 1, H - 2)

    def dn(p):
        dpx = (p[:, ii, jc + 1] - p[:, ii, jc - 1]) / (2.0 * DX)
        dpy = (p[:, ic + 1, jj] - p[:, ic - 1, jj]) / (2.0 * DY)
        return nx[ii, jj] * dpx + ny[ii, jj] * dpy

    mm = EPS1 * dn(p1) - EPS2 * dn(p2)
    tot += float(np.sum(mm * mm))
    return tot


def _prepare(mask):
    np_dt = ml_dtypes.bfloat16
    nx, ny = _normals(H, W)
    boxes, host_cells = _plan(mask)
    classes, stacks, w_tot = _stack(boxes)

    af = 40000.0 * nx
    bf = 40000.0 * ny
    cst = np.zeros((128, 3 * w_tot), dtype=np.float64)
    for bx in boxes:
        rs = slice(bx.r0, bx.r0 + bx.nr)
        cs = slice(bx.c0, bx.c0 + bx.w)
        a = np.where(bx.sel, af[rs, cs], 0.0)
        b = np.where(bx.sel, bf[rs, cs], 0.0)
        a_sh = np.zeros_like(a)
        a_sh[:, :-1] = a[:, 1:]
        b_sh = np.zeros_like(b)
        b_sh[:, :-1] = b[:, 1:]
        ps = slice(bx.part0, bx.part0 + bx.nr)
        cst[ps, bx.f0 : bx.f0 + bx.w] = a_sh
        cst[ps, w_tot + bx.f0 : w_tot + bx.f0 + bx.w] = b_sh
        cst[ps, 2 * w_tot + bx.f0 : 2 * w_tot + bx.f0 + bx.w] = bx.sel

    dmat = np.zeros((128, 128), dtype=np.float64)
    for mi in range(1, 127):
        dmat[mi + 1, mi] = 1.0
        dmat[mi - 1, mi] = -1.0

    consts = {"cst": cst.astype(np_dt), "dmat": dmat.astype(np_dt)}

    # split each class's stacks into halves for DMA/compute overlap
    units = []  # (w, stack_sublist)
    for wq, cstacks in classes:
        if len(cstacks) >= 4:
            mid = (len(cstacks) + 1) // 2
            units.append((wq, cstacks[:mid]))
            units.append((wq, cstacks[mid:]))
        else:
            units.append((wq, cstacks))
    return boxes, units, w_tot, consts, host_cells, np_dt


def _build_nc(units, w_tot):
    from contextlib import ExitStack
    from concourse import bass, bacc, tile, mybir

    mdt = mybir.dt.bfloat16
    f32 = mybir.dt.float32
    mult = mybir.AluOpType.mult
    sub = mybir.AluOpType.subtract
    SQ = mybir.ActivationFunctionType.Square

    F8 = 8 * w_tot
    nu = len(units)

    nc = bacc.Bacc(
        "TRN2", target_bir_lowering=False, debug=False, num_devices=N_CORES
    )
    x_d = nc.dram_tensor("x", [2 * BPC * H, W], mdt, kind="ExternalInput")
    cst_d = nc.dram_tensor("cst", [128, 3 * w_tot], mdt, kind="ExternalInput")
    dmat_d = nc.dram_tensor("dmat", [128, 128], mdt, kind="ExternalInput")
    acc_d = nc.dram_tensor("acc", [128, nu], f32, kind="ExternalOutput")

    with tile.TileContext(nc) as tc, ExitStack() as ctx:
        onep = ctx.enter_context(tc.tile_pool(name="onep", bufs=1))
        dpool = ctx.enter_context(tc.tile_pool(name="dpool", bufs=2))
        vpool = ctx.enter_context(tc.tile_pool(name="vpool", bufs=2))
        pp = ctx.enter_context(tc.tile_pool(name="pp", bufs=2, space="PSUM"))

        X = onep.tile([128, 16 * w_tot], mdt)
        psi = onep.tile([128, F8 + 8], mdt)
        dxs = onep.tile([128, F8], mdt)
        sq = onep.tile([128, 2 * F8], mdt)
        cstt = onep.tile([128, 3 * w_tot], mdt)
        dm = onep.tile([128, 128], mdt)
        acc = onep.tile([128, nu], f32)

        nc.sync.dma_start(cstt[:], cst_d.ap())
        nc.sync.dma_start(dm[:], dmat_d.ap())
        nc.vector.memset(acc[:], 0.0)
        nc.vector.memset(psi[:, F8 : F8 + 8], 0.0)

        # memset empty stack slots of X so psi/df stay finite.  SBUF APs
        # may start only at partition 0/32/64/96 (max 128/32/64/32 rows).
        def memset_parts(a, b, c0, c1):
            while a < b:
                n = {0: 128, 32: 32, 64: 64, 96: 32}[a]
                n = min(n, b - a)
                nc.vector.memset(X[a : a + n, c0:c1], 0.0)
                a += n

        for wq, ustacks in units:
            for st in ustacks:
                used = sum(bx.nr for bx in st)
                if used < 128:
                    f0 = st[0].f0
                    memset_parts(used, 128, 16 * f0, 16 * (f0 + wq))

        # input DMAs in unit order, alternating HWDGE queues (sync-heavy)
        qi = 0
        for wq, ustacks in units:
            for st in ustacks:
                for bx in st:
                    src = bass.AP(
                        x_d,
                        bx.r0 * W + bx.c0,
                        [[W, bx.nr], [H * W, 2 * BPC], [1, bx.w]],
                    )
                    dst = (
                        X[
                            bx.part0 : bx.part0 + bx.nr,
                            16 * bx.f0 : 16 * (bx.f0 + bx.w),
                        ]
                        .rearrange("p (bt w) -> p bt w", bt=2 * BPC)
                    )
                    eng = nc.sync if qi % 3 != 2 else nc.scalar
                    eng.dma_start(dst, src)
                    qi += 1

        def unit_geom(ustacks, wq):
            S = len(ustacks)
            f0 = ustacks[0][0].f0
            wg = S * wq
            ga, gb = 8 * f0, 8 * f0 + 8 * wg
            return S, f0, wg, ga, gb

        def xviews(f0, wg, S):
            xv = X[:, 16 * f0 : 16 * (f0 + wg)].rearrange(
                "p (s b w) -> p s b w", s=S, b=2 * BPC
            )
            return xv[:, :, 0:BPC, :], xv[:, :, BPC : 2 * BPC, :]

        # psi = 0.025*f2 - f1 for every unit up front: the shifted-rhs dy
        # matmul of unit i peeks one column into unit i+1's psi region.
        for wq, ustacks in units:
            S, f0, wg, ga, gb = unit_geom(ustacks, wq)
            xt0, xt1 = xviews(f0, wg, S)
            p4 = psi[:, ga:gb].rearrange("p (s b w) -> p s b w", s=S, b=BPC)
            nc.vector.scalar_tensor_tensor(p4, xt1, 0.025, xt0, op0=mult, op1=sub)

        off = 0  # running col offset into sq: per unit [wt | dfm]
        for ui, (wq, ustacks) in enumerate(units):
            S, f0, wg, ga, gb = unit_geom(ustacks, wq)
            xt0, xt1 = xviews(f0, wg, S)
            # df = f1 - f2  (Pool)
            dft = dpool.tile([128, 8 * wg], mdt, tag="df")
            d4 = dft[:].rearrange("p (s b w) -> p s b w", s=S, b=BPC)
            nc.gpsimd.tensor_sub(d4, xt0, xt1)
            # dfm = df * M
            mview = (
                cstt[:, 2 * w_tot + f0 : 2 * w_tot + f0 + wg]
                .rearrange("p (s w) -> p s w", s=S)
                .unsqueeze(2)
                .broadcast_to([128, S, BPC, wq])
            )
            dfm4 = sq[:, off + 8 * wg : off + 16 * wg].rearrange(
                "p (s b w) -> p s b w", s=S, b=BPC
            )
            eng = nc.gpsimd if wq < 64 else nc.vector
            eng.tensor_mul(dfm4, d4, mview)
            # dxs over the unit's psi range (tail 2 cols masked by A=0)
            nc.vector.tensor_sub(
                dxs[:, ga : gb - 2], psi[:, ga + 2 : gb], psi[:, ga : gb - 2]
            )
            nc.vector.memset(dxs[:, gb - 2 : gb], 0.0)
            # u = A * dxs into sq
            aview = (
                cstt[:, f0 : f0 + wg]
                .rearrange("p (s w) -> p s w", s=S)
                .unsqueeze(2)
                .broadcast_to([128, S, BPC, wq])
            )
            u4 = sq[:, off : off + 8 * wg].rearrange(
                "p (s b w) -> p s b w", s=S, b=BPC
            )
            nc.vector.tensor_mul(
                u4,
                dxs[:, ga:gb].rearrange("p (s b w) -> p s b w", s=S, b=BPC),
                aview,
            )
            # Dy matmuls over this unit's psi range, chunked on the tile's
            # 512 grid so each write stays within one PSUM bank; rhs is
            # shifted +1 col so psum[k] = Dy at cell k+1
            dyp = pp.tile([128, 8 * wg], f32, tag="dy")
            for ca0 in range(0, 8 * wg, 512):
                cb0 = min(ca0 + 512, 8 * wg)
                nc.tensor.matmul(
                    dyp[:, ca0:cb0],
                    dm[:],
                    psi[:, ga + ca0 + 1 : ga + cb0 + 1],
                    start=True,
                    stop=True,
                )
            # v = B * dy
            bview = (
                cstt[:, w_tot + f0 : w_tot + f0 + wg]
                .rearrange("p (s w) -> p s w", s=S)
                .unsqueeze(2)
                .broadcast_to([128, S, BPC, wq])
            )
            vt = vpool.tile([128, 8 * wg], mdt, tag="v")
            nc.vector.tensor_mul(
                vt[:].rearrange("p (s b w) -> p s b w", s=S, b=BPC),
                dyp[:].rearrange("p (s b w) -> p s b w", s=S, b=BPC),
                bview,
            )
            # wt = u + v
            nc.vector.tensor_add(
                sq[:, off : off + 8 * wg], sq[:, off : off + 8 * wg], vt[:]
            )
            # Square+accum over [wt | dfm]; X's region is dead, use as trash
            nc.scalar.activation(
                X[:, 16 * f0 : 16 * (f0 + wg)],
                sq[:, off : off + 16 * wg],
                SQ,
                accum_out=acc[:, ui : ui + 1],
            )
            off += 16 * wg

        nc.sync.dma_start(acc_d.ap(), acc[:])

    nc.compile()
    return nc


_CACHE = {}


def kernel(output_in, output_out, interface_mask):
    from concourse.bass_utils import run_bass_kernel_spmd

    phi1 = np.asarray(output_in).reshape(B, H, W)
    phi2 = np.asarray(output_out).reshape(B, H, W)
    mask = np.asarray(interface_mask).astype(bool)

    n_mask = float(mask.sum())
    if n_mask == 0.0:
        return np.float32(np.nan)

    key = mask.tobytes()
    if key not in _CACHE:
        boxes, units, w_tot, consts, host_cells, np_dt = _prepare(mask)
        nc = _build_nc(units, w_tot) if boxes else None
        _CACHE[key] = (units, w_tot, consts, host_cells, np_dt, nc)
    units, w_tot, consts, host_cells, np_dt, nc = _CACHE[key]

    tot = 0.0
    if nc is not None:
        xi = np.empty((N_CORES, 2, BPC, H, W), dtype=np_dt)
        p1b = phi1.astype(np_dt).reshape(N_CORES, BPC, H, W)
        p2b = phi2.astype(np_dt).reshape(N_CORES, BPC, H, W)
        xi[:, 0] = p1b
        xi[:, 1] = p2b
        in_maps = []
        for c in range(N_CORES):
            m = dict(consts)
            m["x"] = xi[c].reshape(2 * BPC * H, W)
            in_maps.append(m)
        res = run_bass_kernel_spmd(
            nc, in_maps, core_ids=list(range(N_CORES)), trace=TRACE
        )
        global LAST_EXEC_NS
        LAST_EXEC_NS = res.exec_time_ns
        for r in res.results:
            tot += float(r["acc"].astype(np.float64).sum())

    if host_cells.any():
        nx, ny = _normals(H, W)
        tot += _host_contrib(np.nonzero(host_cells), phi1, phi2, nx, ny)

    denom = B * n_mask
    return np.float32(WEIGHT * tot / denom)


# revision 24
# speedup vs baseline: 1.1318x; 1.1318x over previous
"""InterfaceBoundaryLoss Trainium2 kernel (v3).

Data-parallel over batch across 8 NeuronCores.  The [H,W] interface mask is
covered on the host with boxes hugging the circle: 128-row-tall boxes where
the arc is steep, 32- or 16-row bands elsewhere (cost-driven), wide clusters
split into <=64-col pieces.  Short boxes pack into 128-partition stacks of
equal quantized width; stacks of the same width form a "class" processed by
single multi-stack (4D-AP) vector instructions, so the instruction count
stays small.  phi1/phi2 are packed host-side t-major ([2,BPC,H,W]) so each
box loads with one 3D-AP DMA and psi/df are flat subtractions.

Math per mask cell: pot += (phi1-phi2)^2, der += (A*Dx(psi) + B*Dy(psi))^2
with psi = 0.025*phi2 - phi1, A = 40000*m*nx, B = 40000*m*ny shifted one
col left (layout col k = cell k+1).  Dy runs on the TensorEngine via a
banded +/-1 matrix with the moving tensor shifted one col right, so B*dy
aligns with A*Dx without extra shifts; block-halo rows are masked.  The
pot path (df, mask-mul) runs on GpSimd/DVE; one Square+accum activation
per class-half sums pot+der together.  Host sums partials in float64.
"""

import sys

for _p in ("/opt/trn_rl_repo",):
    if _p not in sys.path:
        sys.path.append(_p)

import numpy as np
import ml_dtypes

B, H, W = 64, 1024, 1024
EPS1, EPS2 = 80.0, 2.0
DX, DY = 0.001, 0.001
CX, CY = 512.0, 512.0
WEIGHT = 1.0
N_CORES = 8
BPC = B // N_CORES

TALL = 128
TALL_MAX_W = 64
WQ = (16, 32, 64, 128)
GAP = 6
SUB_W = 124
BOX_PENALTY = 2500  # cells-equivalent cost of one extra box (DMA+sem)

TRACE = False
LAST_EXEC_NS = None


class _Box:
    __slots__ = ("r0", "nr", "c0", "w", "sel", "part0", "f0")

    def __init__(self, r0, nr, c0, w):
        self.r0, self.nr, self.c0, self.w = int(r0), int(nr), int(c0), int(w)
        self.sel = None


def _clusters(cols, gap=GAP):
    out = []
    s = p = cols[0]
    for c in cols[1:]:
        if c - p > gap:
            out.append((s, p))
            s = c
        p = c
    out.append((s, p))
    return out


def _band_pieces(cols):
    """Split a band's mask cols into quantized-width pieces.
    Returns list of (pa, pb, wq, c0)."""
    out = []
    for ca, cb in _clusters(cols):
        span = cb - ca + 1
        net = SUB_W - 4
        npieces = max(1, -(-span // net)) if span > SUB_W - 2 else 1
        for pi in range(npieces):
            pa = ca + pi * net
            pb = min(pa + net - 1, cb)
            if pa > cb:
                break
            ww = pb - pa + 3
            wq = next((q for q in WQ if q >= ww), None)
            if wq is None:
                wq = -(-ww // 64) * 64
            c0 = pa - 1 - (wq - ww) // 2
            c0 = max(0, min(c0, W - wq))
            out.append((pa, pb, wq, c0))
    return out


def _plan(mask):
    h, w_ = mask.shape
    border = np.zeros_like(mask)
    border[0, :] = border[-1, :] = True
    border[:, 0] = border[:, -1] = True
    host_cells = mask & border
    core = mask & ~border
    assigned = np.zeros_like(mask)

    rows_any = np.flatnonzero(core.any(axis=1))
    boxes = []
    if len(rows_any) == 0:
        return boxes, host_cells

    def emit(r, nr):
        own_lo, own_hi = r, min(r + nr - 2, int(rows_any[-1]) + 1)
        sub = core[own_lo:own_hi]
        cols = np.flatnonzero(sub.any(axis=0))
        for pa, pb, wq, c0 in _band_pieces(cols):
            bx = _Box(r - 1, nr, c0, wq)
            sel = np.zeros((nr, wq), dtype=bool)
            s = (
                core[own_lo:own_hi, pa : pb + 1]
                & ~assigned[own_lo:own_hi, pa : pb + 1]
            )
            sel[own_lo - bx.r0 : own_hi - bx.r0, pa - c0 : pb + 1 - c0] = s
            assigned[own_lo:own_hi, pa : pb + 1] |= s
            rr_, cc_ = np.nonzero(sel)
            if len(rr_) == 0:
                continue
            assert rr_.min() >= 1 and rr_.max() <= nr - 2
            assert cc_.min() >= 1 and cc_.max() <= wq - 2
            bx.sel = sel
            boxes.append(bx)
        return own_hi

    def band_cost(r, nr):
        own_lo, own_hi = r, min(r + nr - 2, int(rows_any[-1]) + 1)
        cols = np.flatnonzero(core[own_lo:own_hi].any(axis=0))
        if len(cols) == 0:
            return 0.0, 0
        pieces = _band_pieces(cols)
        return sum(nr * wq + BOX_PENALTY for _, _, wq, _ in pieces), own_hi

    r = int(rows_any[0])
    rmax = int(rows_any[-1])
    while r <= rmax:
        if not core[r].any():
            r += 1
            continue
        # tall band if clusters stay narrow over 126 owned rows
        own_hi = min(r + TALL - 2, rmax + 1)
        cols = np.flatnonzero(core[r:own_hi].any(axis=0))
        cls = _clusters(cols) if len(cols) else []
        if (
            cls
            and max(cb - ca + 1 for ca, cb in cls) <= TALL_MAX_W
            and own_hi - r >= 96
            and r - 1 + TALL <= h
        ):
            r = emit(r, TALL)
            continue
        # otherwise 32-row band (SBUF APs must start at a multiple of 32,
        # so shorter bands cannot pack the partition dim)
        r = emit(r, 32)

    leftover = core & ~assigned
    if leftover.any():
        host_cells = host_cells | leftover
    return boxes, host_cells


def _stack(boxes):
    """Pack boxes into 128-partition stacks of equal width (class = width).
    Talls stand alone.  Returns ordered stack list grouped by class, and
    per-class lists; assigns part0/f0."""
    by_w = {}
    for bx in boxes:
        by_w.setdefault(bx.w, []).append(bx)
    classes = []  # (w, [stacks])
    for wq in sorted(by_w, key=lambda w: -w):
        stacks = []
        cur, used = [], 0
        # first-fit in emit order keeps vertically-adjacent bands together
        for bx in by_w[wq]:
            if used + bx.nr > 128:
                stacks.append(cur)
                cur, used = [], 0
            bx.part0 = used
            cur.append(bx)
            used += bx.nr
        if cur:
            stacks.append(cur)
        classes.append((wq, stacks))
    f = 0
    ordered = []
    for wq, stacks in classes:
        for st in stacks:
            for bx in st:
                bx.f0 = f
            ordered.append(st)
            f += wq
    return classes, ordered, f


def _normals(h, w):
    ii = np.arange(h, dtype=np.float64)[:, None]
    jj = np.arange(w, dtype=np.float64)[None, :]
    nx = jj - CX
    ny = ii - CY
    norm = np.sqrt(nx * nx + ny * ny)
    safe = np.where(norm > 0, norm, 1.0)
    return nx / safe, ny / safe


def _host_contrib(cells_ij, phi1, phi2, nx, ny):
    if len(cells_ij[0]) == 0:
        return 0.0
    ii, jj = cells_ij
    p1 = phi1.astype(np.float64)
    p2 = phi2.astype(np.float64)
    d = p1[:, ii, jj] - p2[:, ii, jj]
    tot = float(np.sum(d * d))
    jc = np.clip(jj, 1, W - 2)
    ic = np.clip(ii, 1, H - 2)

    def dn(p):
        dpx = (p[:, ii, jc + 1] - p[:, ii, jc - 1]) / (2.0 * DX)
        dpy = (p[:, ic + 1, jj] - p[:, ic - 1, jj]) / (2.0 * DY)
        return nx[ii, jj] * dpx + ny[ii, jj] * dpy

    mm = EPS1 * dn(p1) - EPS2 * dn(p2)
    tot += float(np.sum(mm * mm))
    return tot


def _prepare(mask):
    np_dt = ml_dtypes.bfloat16
    nx, ny = _normals(H, W)
    boxes, host_cells = _plan(mask)
    classes, stacks, w_tot = _stack(boxes)

    af = 40000.0 * nx
    bf = 40000.0 * ny
    cst = np.zeros((128, 3 * w_tot), dtype=np.float64)
    for bx in boxes:
        rs = slice(bx.r0, bx.r0 + bx.nr)
        cs = slice(bx.c0, bx.c0 + bx.w)
        a = np.where(bx.sel, af[rs, cs], 0.0)
        b = np.where(bx.sel, bf[rs, cs], 0.0)
        a_sh = np.zeros_like(a)
        a_sh[:, :-1] = a[:, 1:]
        b_sh = np.zeros_like(b)
        b_sh[:, :-1] = b[:, 1:]
        ps = slice(bx.part0, bx.part0 + bx.nr)
        cst[ps, bx.f0 : bx.f0 + bx.w] = a_sh
        cst[ps, w_tot + bx.f0 : w_tot + bx.f0 + bx.w] = b_sh
        cst[ps, 2 * w_tot + bx.f0 : 2 * w_tot + bx.f0 + bx.w] = bx.sel

    dmat = np.zeros((128, 128), dtype=np.float64)
    for mi in range(1, 127):
        dmat[mi + 1, mi] = 1.0
        dmat[mi - 1, mi] = -1.0

    consts = {"cst": cst.astype(np_dt), "dmat": dmat.astype(np_dt)}

    # split each class's stacks into halves for DMA/compute overlap
    units = []  # (w, stack_sublist)
    for wq, cstacks in classes:
        if len(cstacks) >= 4:
            mid = (len(cstacks) + 1) // 2
            units.append((wq, cstacks[:mid]))
            units.append((wq, cstacks[mid:]))
        else:
            units.append((wq, cstacks))
    return boxes, units, w_tot, consts, host_cells, np_dt


def _build_nc(units, w_tot):
    from contextlib import ExitStack
    from concourse import bass, bacc, tile, mybir

    mdt = mybir.dt.bfloat16
    f32 = mybir.dt.float32
    mult = mybir.AluOpType.mult
    sub = mybir.AluOpType.subtract
    SQ = mybir.ActivationFunctionType.Square

    F8 = 8 * w_tot
    nu = len(units)

    nc = bacc.Bacc(
        "TRN2", target_bir_lowering=False, debug=False, num_devices=N_CORES
    )
    x_d = nc.dram_tensor("x", [2 * BPC * H, W], mdt, kind="ExternalInput")
    cst_d = nc.dram_tensor("cst", [128, 3 * w_tot], mdt, kind="ExternalInput")
    dmat_d = nc.dram_tensor("dmat", [128, 128], mdt, kind="ExternalInput")
    acc_d = nc.dram_tensor("acc", [128, nu], f32, kind="ExternalOutput")

    with tile.TileContext(nc) as tc, ExitStack() as ctx:
        onep = ctx.enter_context(tc.tile_pool(name="onep", bufs=1))
        dpool = ctx.enter_context(tc.tile_pool(name="dpool", bufs=2))
        vpool = ctx.enter_context(tc.tile_pool(name="vpool", bufs=2))
        pp = ctx.enter_context(tc.tile_pool(name="pp", bufs=2, space="PSUM"))

        X = onep.tile([128, 16 * w_tot], mdt)
        psi = onep.tile([128, F8 + 8], mdt)
        dxs = onep.tile([128, F8], mdt)
        sq = onep.tile([128, 2 * F8], mdt)
        cstt = onep.tile([128, 3 * w_tot], mdt)
        dm = onep.tile([128, 128], mdt)
        acc = onep.tile([128, nu], f32)

        nc.scalar.dma_start(cstt[:], cst_d.ap())
        nc.scalar.dma_start(dm[:], dmat_d.ap())
        nc.vector.memset(acc[:], 0.0)
        nc.vector.memset(psi[:, F8 : F8 + 8], 0.0)

        # memset empty stack slots of X so psi/df stay finite.  SBUF APs
        # may start only at partition 0/32/64/96 (max 128/32/64/32 rows).
        def memset_parts(a, b, c0, c1):
            while a < b:
                n = {0: 128, 32: 32, 64: 64, 96: 32}[a]
                n = min(n, b - a)
                nc.vector.memset(X[a : a + n, c0:c1], 0.0)
                a += n

        for wq, ustacks in units:
            for st in ustacks:
                used = sum(bx.nr for bx in st)
                if used < 128:
                    f0 = st[0].f0
                    memset_parts(used, 128, 16 * f0, 16 * (f0 + wq))

        # input DMAs in unit order, alternating HWDGE queues (sync-heavy)
        qi = 0
        for wq, ustacks in units:
            for st in ustacks:
                for bx in st:
                    src = bass.AP(
                        x_d,
                        bx.r0 * W + bx.c0,
                        [[W, bx.nr], [H * W, 2 * BPC], [1, bx.w]],
                    )
                    dst = (
                        X[
                            bx.part0 : bx.part0 + bx.nr,
                            16 * bx.f0 : 16 * (bx.f0 + bx.w),
                        ]
                        .rearrange("p (bt w) -> p bt w", bt=2 * BPC)
                    )
                    eng = nc.sync if qi % 2 == 0 else nc.scalar
                    eng.dma_start(dst, src)
                    qi += 1

        def unit_geom(ustacks, wq):
            S = len(ustacks)
            f0 = ustacks[0][0].f0
            wg = S * wq
            ga, gb = 8 * f0, 8 * f0 + 8 * wg
            return S, f0, wg, ga, gb

        def xviews(f0, wg, S):
            xv = X[:, 16 * f0 : 16 * (f0 + wg)].rearrange(
                "p (s b w) -> p s b w", s=S, b=2 * BPC
            )
            return xv[:, :, 0:BPC, :], xv[:, :, BPC : 2 * BPC, :]

        def emit_psi(u):
            wq, ustacks = units[u]
            S, f0, wg, ga, gb = unit_geom(ustacks, wq)
            xt0, xt1 = xviews(f0, wg, S)
            p4 = psi[:, ga:gb].rearrange("p (s b w) -> p s b w", s=S, b=BPC)
            nc.vector.scalar_tensor_tensor(p4, xt1, 0.025, xt0, op0=mult, op1=sub)

        # psi is emitted one unit ahead: the shifted-rhs dy matmul of unit
        # i peeks one column into unit i+1's psi region, so psi(i+1) must
        # precede unit i's matmul without stalling the whole pipeline.
        emit_psi(0)
        off = 0  # running col offset into sq: per unit [wt | dfm]
        for ui, (wq, ustacks) in enumerate(units):
            if ui + 1 < nu:
                emit_psi(ui + 1)
            S, f0, wg, ga, gb = unit_geom(ustacks, wq)
            xt0, xt1 = xviews(f0, wg, S)
            # df = f1 - f2  (Pool)
            dft = dpool.tile([128, 8 * wg], mdt, tag="df")
            d4 = dft[:].rearrange("p (s b w) -> p s b w", s=S, b=BPC)
            nc.gpsimd.tensor_sub(d4, xt0, xt1)
            # dfm = df * M
            mview = (
                cstt[:, 2 * w_tot + f0 : 2 * w_tot + f0 + wg]
                .rearrange("p (s w) -> p s w", s=S)
                .unsqueeze(2)
                .broadcast_to([128, S, BPC, wq])
            )
            dfm4 = sq[:, off + 8 * wg : off + 16 * wg].rearrange(
                "p (s b w) -> p s b w", s=S, b=BPC
            )
            eng = nc.gpsimd if wq < 64 else nc.vector
            eng.tensor_mul(dfm4, d4, mview)
            # dxs over the unit's psi range (tail 2 cols masked by A=0)
            nc.vector.tensor_sub(
                dxs[:, ga : gb - 2], psi[:, ga + 2 : gb], psi[:, ga : gb - 2]
            )
            nc.vector.memset(dxs[:, gb - 2 : gb], 0.0)
            # u = A * dxs into sq
            aview = (
                cstt[:, f0 : f0 + wg]
                .rearrange("p (s w) -> p s w", s=S)
                .unsqueeze(2)
                .broadcast_to([128, S, BPC, wq])
            )
            u4 = sq[:, off : off + 8 * wg].rearrange(
                "p (s b w) -> p s b w", s=S, b=BPC
            )
            nc.vector.tensor_mul(
                u4,
                dxs[:, ga:gb].rearrange("p (s b w) -> p s b w", s=S, b=BPC),
                aview,
            )
            # Dy matmuls over this unit's psi range, chunked on the tile's
            # 512 grid so each write stays within one PSUM bank; rhs is
            # shifted +1 col so psum[k] = Dy at cell k+1
            dyp = pp.tile([128, 8 * wg], f32, tag="dy")
            for ca0 in range(0, 8 * wg, 512):
                cb0 = min(ca0 + 512, 8 * wg)
                nc.tensor.matmul(
                    dyp[:, ca0:cb0],
                    dm[:],
                    psi[:, ga + ca0 + 1 : ga + cb0 + 1],
                    start=True,
                    stop=True,
                )
            # v = B * dy
            bview = (
                cstt[:, w_tot + f0 : w_tot + f0 + wg]
                .rearrange("p (s w) -> p s w", s=S)
                .unsqueeze(2)
                .broadcast_to([128, S, BPC, wq])
            )
            vt = vpool.tile([128, 8 * wg], mdt, tag="v")
            nc.vector.tensor_mul(
                vt[:].rearrange("p (s b w) -> p s b w", s=S, b=BPC),
                dyp[:].rearrange("p (s b w) -> p s b w", s=S, b=BPC),
                bview,
            )
            # wt = u + v
            nc.vector.tensor_add(
                sq[:, off : off + 8 * wg], sq[:, off : off + 8 * wg], vt[:]
            )
            # Square+accum over [wt | dfm]; X's region is dead, use as trash
            nc.scalar.activation(
                X[:, 16 * f0 : 16 * (f0 + wg)],
                sq[:, off : off + 16 * wg],
                SQ,
                accum_out=acc[:, ui : ui + 1],
            )
            off += 16 * wg

        nc.sync.dma_start(acc_d.ap(), acc[:])

    nc.compile()
    return nc


_CACHE = {}


def kernel(output_in, output_out, interface_mask):
    from concourse.bass_utils import run_bass_kernel_spmd

    phi1 = np.asarray(output_in).reshape(B, H, W)
    phi2 = np.asarray(output_out).reshape(B, H, W)
    mask = np.asarray(interface_mask).astype(bool)

    n_mask = float(mask.sum())
    if n_mask == 0.0:
        return np.float32(np.nan)

    key = mask.tobytes()
    if key not in _CACHE:
        boxes, units, w_tot, consts, host_cells, np_dt = _prepare(mask)
        nc = _build_nc(units, w_tot) if boxes else None
        _CACHE[key] = (units, w_tot, consts, host_cells, np_dt, nc)
    units, w_tot, consts, host_cells, np_dt, nc = _CACHE[key]

    tot = 0.0
    if nc is not None:
        xi = np.empty((N_CORES, 2, BPC, H, W), dtype=np_dt)
        p1b = phi1.astype(np_dt).reshape(N_CORES, BPC, H, W)
        p2b = phi2.astype(np_dt).reshape(N_CORES, BPC, H, W)
        xi[:, 0] = p1b
        xi[:, 1] = p2b
        in_maps = []
        for c in range(N_CORES):
            m = dict(consts)
            m["x"] = xi[c].reshape(2 * BPC * H, W)
            in_maps.append(m)
        res = run_bass_kernel_spmd(
            nc, in_maps, core_ids=list(range(N_CORES)), trace=TRACE
        )
        global LAST_EXEC_NS
        LAST_EXEC_NS = res.exec_time_ns
        for r in res.results:
            tot += float(r["acc"].astype(np.float64).sum())

    if host_cells.any():
        nx, ny = _normals(H, W)
        tot += _host_contrib(np.nonzero(host_cells), phi1, phi2, nx, ny)

    denom = B * n_mask
    return np.float32(WEIGHT * tot / denom)


# revision 29
# speedup vs baseline: 1.1432x; 1.0100x over previous
"""InterfaceBoundaryLoss Trainium2 kernel (v3).

Data-parallel over batch across 8 NeuronCores.  The [H,W] interface mask is
covered on the host with boxes hugging the circle: 128-row-tall boxes where
the arc is steep, 32- or 16-row bands elsewhere (cost-driven), wide clusters
split into <=64-col pieces.  Short boxes pack into 128-partition stacks of
equal quantized width; stacks of the same width form a "class" processed by
single multi-stack (4D-AP) vector instructions, so the instruction count
stays small.  phi1/phi2 are packed host-side t-major ([2,BPC,H,W]) so each
box loads with one 3D-AP DMA and psi/df are flat subtractions.

Math per mask cell: pot += (phi1-phi2)^2, der += (A*Dx(psi) + B*Dy(psi))^2
with psi = 0.025*phi2 - phi1, A = 40000*m*nx, B = 40000*m*ny shifted one
col left (layout col k = cell k+1).  Dy runs on the TensorEngine via a
banded +/-1 matrix with the moving tensor shifted one col right, so B*dy
aligns with A*Dx without extra shifts; block-halo rows are masked.  The
pot path (df, mask-mul) runs on GpSimd/DVE; one Square+accum activation
per class-half sums pot+der together.  Host sums partials in float64.
"""

import sys

for _p in ("/opt/trn_rl_repo",):
    if _p not in sys.path:
        sys.path.append(_p)

import numpy as np
import ml_dtypes

B, H, W = 64, 1024, 1024
EPS1, EPS2 = 80.0, 2.0
DX, DY = 0.001, 0.001
CX, CY = 512.0, 512.0
WEIGHT = 1.0
N_CORES = 8
BPC = B // N_CORES

TALL = 128
TALL_MAX_W = 64
WQ = (16, 32, 64, 128)
GAP = 6
SUB_W = 124
BOX_PENALTY = 2500  # cells-equivalent cost of one extra box (DMA+sem)

TRACE = False
LAST_EXEC_NS = None


class _Box:
    __slots__ = ("r0", "nr", "c0", "w", "sel", "part0", "f0")

    def __init__(self, r0, nr, c0, w):
        self.r0, self.nr, self.c0, self.w = int(r0), int(nr), int(c0), int(w)
        self.sel = None


def _clusters(cols, gap=GAP):
    out = []
    s = p = cols[0]
    for c in cols[1:]:
        if c - p > gap:
            out.append((s, p))
            s = c
        p = c
    out.append((s, p))
    return out


def _band_pieces(cols):
    """Split a band's mask cols into quantized-width pieces.
    Returns list of (pa, pb, wq, c0)."""
    out = []
    for ca, cb in _clusters(cols):
        span = cb - ca + 1
        net = SUB_W - 4
        npieces = max(1, -(-span // net)) if span > SUB_W - 2 else 1
        for pi in range(npieces):
            pa = ca + pi * net
            pb = min(pa + net - 1, cb)
            if pa > cb:
                break
            ww = pb - pa + 3
            wq = next((q for q in WQ if q >= ww), None)
            if wq is None:
                wq = -(-ww // 64) * 64
            c0 = pa - 1 - (wq - ww) // 2
            c0 = max(0, min(c0, W - wq))
            out.append((pa, pb, wq, c0))
    return out


def _plan(mask):
    h, w_ = mask.shape
    border = np.zeros_like(mask)
    border[0, :] = border[-1, :] = True
    border[:, 0] = border[:, -1] = True
    host_cells = mask & border
    core = mask & ~border
    assigned = np.zeros_like(mask)

    rows_any = np.flatnonzero(core.any(axis=1))
    boxes = []
    if len(rows_any) == 0:
        return boxes, host_cells

    def emit(r, nr):
        own_lo, own_hi = r, min(r + nr - 2, int(rows_any[-1]) + 1)
        sub = core[own_lo:own_hi]
        cols = np.flatnonzero(sub.any(axis=0))
        for pa, pb, wq, c0 in _band_pieces(cols):
            bx = _Box(r - 1, nr, c0, wq)
            sel = np.zeros((nr, wq), dtype=bool)
            s = (
                core[own_lo:own_hi, pa : pb + 1]
                & ~assigned[own_lo:own_hi, pa : pb + 1]
            )
            sel[own_lo - bx.r0 : own_hi - bx.r0, pa - c0 : pb + 1 - c0] = s
            assigned[own_lo:own_hi, pa : pb + 1] |= s
            rr_, cc_ = np.nonzero(sel)
            if len(rr_) == 0:
                continue
            assert rr_.min() >= 1 and rr_.max() <= nr - 2
            assert cc_.min() >= 1 and cc_.max() <= wq - 2
            bx.sel = sel
            boxes.append(bx)
        return own_hi

    def band_cost(r, nr):
        own_lo, own_hi = r, min(r + nr - 2, int(rows_any[-1]) + 1)
        cols = np.flatnonzero(core[own_lo:own_hi].any(axis=0))
        if len(cols) == 0:
            return 0.0, 0
        pieces = _band_pieces(cols)
        return sum(nr * wq + BOX_PENALTY for _, _, wq, _ in pieces), own_hi

    r = int(rows_any[0])
    rmax = int(rows_any[-1])
    while r <= rmax:
        if not core[r].any():
            r += 1
            continue
        # tall band if clusters stay narrow over 126 owned rows
        own_hi = min(r + TALL - 2, rmax + 1)
        cols = np.flatnonzero(core[r:own_hi].any(axis=0))
        cls = _clusters(cols) if len(cols) else []
        if (
            cls
            and max(cb - ca + 1 for ca, cb in cls) <= TALL_MAX_W
            and own_hi - r >= 96
            and r - 1 + TALL <= h
        ):
            r = emit(r, TALL)
            continue
        # otherwise 32-row band (SBUF APs must start at a multiple of 32,
        # so shorter bands cannot pack the partition dim)
        r = emit(r, 32)

    leftover = core & ~assigned
    if leftover.any():
        host_cells = host_cells | leftover
    return boxes, host_cells


def _stack(boxes):
    """Pack boxes into 128-partition stacks of equal width (class = width).
    Talls stand alone.  Returns ordered stack list grouped by class, and
    per-class lists; assigns part0/f0."""
    by_w = {}
    for bx in boxes:
        by_w.setdefault(bx.w, []).append(bx)
    classes = []  # (w, [stacks])
    for wq in sorted(by_w, key=lambda w: -w):
        stacks = []
        cur, used = [], 0
        # first-fit in emit order keeps vertically-adjacent bands together
        for bx in by_w[wq]:
            if used + bx.nr > 128:
                stacks.append(cur)
                cur, used = [], 0
            bx.part0 = used
            cur.append(bx)
            used += bx.nr
        if cur:
            stacks.append(cur)
        classes.append((wq, stacks))
    f = 0
    ordered = []
    for wq, stacks in classes:
        for st in stacks:
            for bx in st:
                bx.f0 = f
            ordered.append(st)
            f += wq
    return classes, ordered, f


def _normals(h, w):
    ii = np.arange(h, dtype=np.float64)[:, None]
    jj = np.arange(w, dtype=np.float64)[None, :]
    nx = jj - CX
    ny = ii - CY
    norm = np.sqrt(nx * nx + ny * ny)
    safe = np.where(norm > 0, norm, 1.0)
    return nx / safe, ny / safe


def _host_contrib(cells_ij, phi1, phi2, nx, ny):
    if len(cells_ij[0]) == 0:
        return 0.0
    ii, jj = cells_ij
    p1 = phi1.astype(np.float64)
    p2 = phi2.astype(np.float64)
    d = p1[:, ii, jj] - p2[:, ii, jj]
    tot = float(np.sum(d * d))
    jc = np.clip(jj, 1, W - 2)
    ic = np.clip(ii, 1, H - 2)

    def dn(p):
        dpx = (p[:, ii, jc + 1] - p[:, ii, jc - 1]) / (2.0 * DX)
        dpy = (p[:, ic + 1, jj] - p[:, ic - 1, jj]) / (2.0 * DY)
        return nx[ii, jj] * dpx + ny[ii, jj] * dpy

    mm = EPS1 * dn(p1) - EPS2 * dn(p2)
    tot += float(np.sum(mm * mm))
    return tot


def _prepare(mask):
    np_dt = ml_dtypes.bfloat16
    nx, ny = _normals(H, W)
    boxes, host_cells = _plan(mask)
    classes, stacks, w_tot = _stack(boxes)

    af = 40000.0 * nx
    bf = 40000.0 * ny
    cst = np.zeros((128, 3 * w_tot), dtype=np.float64)
    for bx in boxes:
        rs = slice(bx.r0, bx.r0 + bx.nr)
        cs = slice(bx.c0, bx.c0 + bx.w)
        a = np.where(bx.sel, af[rs, cs], 0.0)
        b = np.where(bx.sel, bf[rs, cs], 0.0)
        a_sh = np.zeros_like(a)
        a_sh[:, :-1] = a[:, 1:]
        b_sh = np.zeros_like(b)
        b_sh[:, :-1] = b[:, 1:]
        ps = slice(bx.part0, bx.part0 + bx.nr)
        cst[ps, bx.f0 : bx.f0 + bx.w] = a_sh
        cst[ps, w_tot + bx.f0 : w_tot + bx.f0 + bx.w] = b_sh
        cst[ps, 2 * w_tot + bx.f0 : 2 * w_tot + bx.f0 + bx.w] = bx.sel

    dmat = np.zeros((128, 128), dtype=np.float64)
    for mi in range(1, 127):
        dmat[mi + 1, mi] = 1.0
        dmat[mi - 1, mi] = -1.0

    consts = {"cst": cst.astype(np_dt), "dmat": dmat.astype(np_dt)}

    # split each class's stacks into halves for DMA/compute overlap
    units = []  # (w, stack_sublist)
    for wq, cstacks in classes:
        if len(cstacks) >= 4:
            mid = (len(cstacks) + 1) // 2
            units.append((wq, cstacks[:mid]))
            units.append((wq, cstacks[mid:]))
        else:
            units.append((wq, cstacks))
    return boxes, units, w_tot, consts, host_cells, np_dt


def _build_nc(units, w_tot):
    from contextlib import ExitStack
    from concourse import bass, bacc, tile, mybir

    mdt = mybir.dt.bfloat16
    f32 = mybir.dt.float32
    mult = mybir.AluOpType.mult
    sub = mybir.AluOpType.subtract
    SQ = mybir.ActivationFunctionType.Square

    F8 = 8 * w_tot
    nu = len(units)

    nc = bacc.Bacc(
        "TRN2", target_bir_lowering=False, debug=False, num_devices=N_CORES
    )
    x_d = nc.dram_tensor("x", [2 * BPC * H, W], mdt, kind="ExternalInput")
    cst_d = nc.dram_tensor("cst", [128, 3 * w_tot], mdt, kind="ExternalInput")
    dmat_d = nc.dram_tensor("dmat", [128, 128], mdt, kind="ExternalInput")
    acc_d = nc.dram_tensor("acc", [128, nu], f32, kind="ExternalOutput")

    with tile.TileContext(nc) as tc, ExitStack() as ctx:
        onep = ctx.enter_context(tc.tile_pool(name="onep", bufs=1))
        dpool = ctx.enter_context(tc.tile_pool(name="dpool", bufs=2))
        vpool = ctx.enter_context(tc.tile_pool(name="vpool", bufs=2))
        pp = ctx.enter_context(tc.tile_pool(name="pp", bufs=2, space="PSUM"))

        X = onep.tile([128, 16 * w_tot], mdt)
        psi = onep.tile([128, F8 + 8], mdt)
        dxs = onep.tile([128, F8], mdt)
        sq = onep.tile([128, 2 * F8], mdt)
        cstt = onep.tile([128, 3 * w_tot], mdt)
        dm = onep.tile([128, 128], mdt)
        acc = onep.tile([128, nu], f32)

        nc.vector.memset(acc[:], 0.0)
        nc.vector.memset(psi[:, F8 : F8 + 8], 0.0)

        # Per-unit X free layout is (t, b, s, w): batch outermost so the
        # broadcast const operands of u/v/dfm have stride-0 only on the
        # outer free dim with long contiguous inner runs.
        def box_dst(bx, S, si):
            base = 16 * bx.f0 - 16 * si * bx.w  # unit base col
            region = X[
                bx.part0 : bx.part0 + bx.nr,
                base : base + 16 * S * bx.w,
            ].rearrange("p (bt s w) -> p bt s w", bt=2 * BPC, s=S)
            return region[:, :, si : si + 1, :]

        # memset empty stack slots of X so psi/df stay finite.  SBUF APs
        # may start only at partition 0/32/64/96 (max 128/32/64/32 rows).
        def memset_parts(bx, S, si, a, b):
            base = 16 * bx.f0 - 16 * si * bx.w
            while a < b:
                n = {0: 128, 32: 32, 64: 64, 96: 32}[a]
                n = min(n, b - a)
                region = X[
                    a : a + n, base : base + 16 * S * bx.w
                ].rearrange("p (bt s w) -> p bt s w", bt=2 * BPC, s=S)
                nc.vector.memset(region[:, :, si : si + 1, :], 0.0)
                a += n

        for wq, ustacks in units:
            S = len(ustacks)
            for si, st in enumerate(ustacks):
                used = sum(bx.nr for bx in st)
                if used < 128:
                    memset_parts(st[0], S, si, used, 128)

        # input DMAs in unit order, alternating HWDGE queues; the big
        # constant block is issued after the first units' boxes so it
        # doesn't delay the pipeline head on the scalar ring.
        qi = 0
        for wq, ustacks in units:
            S = len(ustacks)
            for si, st in enumerate(ustacks):
                for bx in st:
                    src = bass.AP(
                        x_d,
                        bx.r0 * W + bx.c0,
                        [[W, bx.nr], [H * W, 2 * BPC], [1, bx.w]],
                    )
                    eng = nc.sync if qi % 2 == 0 else nc.scalar
                    eng.dma_start(box_dst(bx, S, si), src)
                    qi += 1
                    if qi == 5:
                        nc.scalar.dma_start(dm[:], dmat_d.ap())
                        nc.scalar.dma_start(cstt[:], cst_d.ap())
        if qi <= 5:
            nc.scalar.dma_start(dm[:], dmat_d.ap())
            nc.scalar.dma_start(cstt[:], cst_d.ap())

        def unit_geom(ustacks, wq):
            S = len(ustacks)
            f0 = ustacks[0][0].f0
            wg = S * wq
            ga, gb = 8 * f0, 8 * f0 + 8 * wg
            return S, f0, wg, ga, gb

        def xviews(f0, wg, S):
            # unit X layout (t, b, s, w): halves are flat [p, 1, 8*wg]
            xv = X[:, 16 * f0 : 16 * (f0 + wg)].rearrange(
                "p (t f) -> p t f", t=2
            )
            return xv[:, 0:1, :], xv[:, 1:2, :]

        def emit_psi(u):
            wq, ustacks = units[u]
            S, f0, wg, ga, gb = unit_geom(ustacks, wq)
            xt0, xt1 = xviews(f0, wg, S)
            pview = psi[:, ga:gb].unsqueeze(1)
            nc.vector.scalar_tensor_tensor(pview, xt1, 0.025, xt0, op0=mult, op1=sub)

        # psi is emitted one unit ahead: the shifted-rhs dy matmul of unit
        # i peeks one column into unit i+1's psi region, so psi(i+1) must
        # precede unit i's matmul without stalling the whole pipeline.
        emit_psi(0)
        off = 0  # running col offset into sq: per unit [wt | dfm]
        for ui, (wq, ustacks) in enumerate(units):
            if ui + 1 < nu:
                emit_psi(ui + 1)
            S, f0, wg, ga, gb = unit_geom(ustacks, wq)
            xt0, xt1 = xviews(f0, wg, S)
            # df = f1 - f2  (Pool)
            dft = dpool.tile([128, 8 * wg], mdt, tag="df")
            nc.gpsimd.tensor_sub(dft[:].unsqueeze(1), xt0, xt1)
            # dfm = df * M  (engine alternates per unit to balance load)
            mview = (
                cstt[:, 2 * w_tot + f0 : 2 * w_tot + f0 + wg]
                .unsqueeze(1)
                .broadcast_to([128, BPC, wg])
            )
            dfm4 = sq[:, off + 8 * wg : off + 16 * wg].rearrange(
                "p (b f) -> p b f", b=BPC
            )
            eng = nc.gpsimd if ui % 2 == 0 else nc.vector
            eng.tensor_mul(
                dfm4, dft[:].rearrange("p (b f) -> p b f", b=BPC), mview
            )
            # dxs over the unit's psi range (tail 2 cols masked by A=0)
            nc.vector.tensor_sub(
                dxs[:, ga : gb - 2], psi[:, ga + 2 : gb], psi[:, ga : gb - 2]
            )
            nc.vector.memset(dxs[:, gb - 2 : gb], 0.0)
            # u = A * dxs into sq
            aview = (
                cstt[:, f0 : f0 + wg]
                .unsqueeze(1)
                .broadcast_to([128, BPC, wg])
            )
            u4 = sq[:, off : off + 8 * wg].rearrange("p (b f) -> p b f", b=BPC)
            nc.vector.tensor_mul(
                u4,
                dxs[:, ga:gb].rearrange("p (b f) -> p b f", b=BPC),
                aview,
            )
            # Dy matmuls over this unit's psi range, chunked on the tile's
            # 512 grid so each write stays within one PSUM bank; rhs is
            # shifted +1 col so psum[k] = Dy at cell k+1
            dyp = pp.tile([128, 8 * wg], f32, tag="dy")
            for ca0 in range(0, 8 * wg, 512):
                cb0 = min(ca0 + 512, 8 * wg)
                nc.tensor.matmul(
                    dyp[:, ca0:cb0],
                    dm[:],
                    psi[:, ga + ca0 + 1 : ga + cb0 + 1],
                    start=True,
                    stop=True,
                )
            # v = B * dy
            bview = (
                cstt[:, w_tot + f0 : w_tot + f0 + wg]
                .unsqueeze(1)
                .broadcast_to([128, BPC, wg])
            )
            vt = vpool.tile([128, 8 * wg], mdt, tag="v")
            nc.vector.tensor_mul(
                vt[:].rearrange("p (b f) -> p b f", b=BPC),
                dyp[:].rearrange("p (b f) -> p b f", b=BPC),
                bview,
            )
            # wt = u + v
            nc.vector.tensor_add(
                sq[:, off : off + 8 * wg], sq[:, off : off + 8 * wg], vt[:]
            )
            # Square+accum over [wt | dfm]; X's region is dead, use as trash
            nc.scalar.activation(
                X[:, 16 * f0 : 16 * (f0 + wg)],
                sq[:, off : off + 16 * wg],
                SQ,
                accum_out=acc[:, ui : ui + 1],
            )
            off += 16 * wg

        nc.sync.dma_start(acc_d.ap(), acc[:])

    nc.compile()
    return nc


_CACHE = {}


def kernel(output_in, output_out, interface_mask):
    from concourse.bass_utils import run_bass_kernel_spmd

    phi1 = np.asarray(output_in).reshape(B, H, W)
    phi2 = np.asarray(output_out).reshape(B, H, W)
    mask = np.asarray(interface_mask).astype(bool)

    n_mask = float(mask.sum())
    if n_mask == 0.0:
        return np.float32(np.nan)

    key = mask.tobytes()
    if key not in _CACHE:
        boxes, units, w_tot, consts, host_cells, np_dt = _prepare(mask)
        nc = _build_nc(units, w_tot) if boxes else None
        _CACHE[key] = (units, w_tot, consts, host_cells, np_dt, nc)
    units, w_tot, consts, host_cells, np_dt, nc = _CACHE[key]

    tot = 0.0
    if nc is not None:
        xi = np.empty((N_CORES, 2, BPC, H, W), dtype=np_dt)
        p1b = phi1.astype(np_dt).reshape(N_CORES, BPC, H, W)
        p2b = phi2.astype(np_dt).reshape(N_CORES, BPC, H, W)
        xi[:, 0] = p1b
        xi[:, 1] = p2b
        in_maps = []
        for c in range(N_CORES):
            m = dict(consts)
            m["x"] = xi[c].reshape(2 * BPC * H, W)
            in_maps.append(m)
        res = run_bass_kernel_spmd(
            nc, in_maps, core_ids=list(range(N_CORES)), trace=TRACE
        )
        global LAST_EXEC_NS
        LAST_EXEC_NS = res.exec_time_ns
        for r in res.results:
            tot += float(r["acc"].astype(np.float64).sum())

    if host_cells.any():
        nx, ny = _normals(H, W)
        tot += _host_contrib(np.nonzero(host_cells), phi1, phi2, nx, ny)

    denom = B * n_mask
    return np.float32(WEIGHT * tot / denom)
